# revision 1
# baseline (speedup 1.0000x reference)
"""Trainium2 Bass kernel: nn_BinaryCQV_End2End (batched 10-qubit circuit sim).

Self-contained: host precompute (angle tables, layer-1 product-state vectors,
folded head/observable weight vector), Bass module builder, 8-core SPMD runner.

Layout: batch on partitions (128/tile, 2 tiles/core, 8 cores = 2048 samples).
State: one (128, 2048) f32 SBUF buffer per tile = [real(1024) | imag(1024)],
qubit 0 = MSB of the 10-bit state index.

Per rotation gate (per-partition scalars c, s from the param table):
  TS : T = swap_q(S) * s           (1 instr, 2048 elems, bit-q-reversed read;
                                    RX additionally swaps the r/i planes)
  STT: S = (S * c) -/+ T           (2 instrs, 1024 elems each; RY splits by
                                    bit q with sub/add, RX by plane add/sub)
CNOT(c,c+1): materialize the permuted state into the scratch buffer with 2
copies of 1024 (identity half + bit-flipped half), then swap buffer roles.
Layer-3's trailing CNOT chain is folded into the weight vector on the host.
Head (incl. bias via sum(p)=1, logit scale) folds into W; device ends with
probs + one weighted reduce. Final clamp happens on the host.
"""
import numpy as np

NQ = 10
NSTATE = 1 << NQ  # 1024
INPUT_DIM = 49
ENC_LAMBDA = float(np.pi)
N_CORES = 8
TILES = 2           # batch tiles per core
P = 128             # partitions
BATCH = N_CORES * TILES * P
PCOLS = 184
LOGIT_SCALE_MIN, LOGIT_SCALE_MAX = 0.5, 80.0
LOGIT_CLAMP = 30.0

_CACHE = {}

# Build-time feature flags (bisect/debug): mutate before first kernel() call.
OPTS = {
    "divert": 0,        # Pool-TT diversion hurts: slow TT sits on the critical path
    "pool_copies": True, # CNOT copies alternate ACT/Pool (False: ACT only)
    "triples": True,     # CNOT chain via triple-folds (False: pairs+single)
}


# ---------------------------------------------------------------- host side
def _softplus(x):
    return np.log1p(np.exp(-np.abs(x))) + np.maximum(x, 0.0)


def host_precompute(x, theta, enc_alpha_raw, enc_beta_raw, head_w, head_b,
                    logit_scale):
    """params (B, 120) f32 and folded weight vector W (1024,) f32."""
    B = x.shape[0]
    alpha = _softplus(np.asarray(enc_alpha_raw, np.float64)) + 1e-6
    beta = np.tanh(np.asarray(enc_beta_raw, np.float64))
    x = np.asarray(x, np.float64)
    theta = np.asarray(theta, np.float64)

    enc_h = np.empty((B, 3, NQ), np.float64)  # ENC half-angles
    for b in range(3):
        for q in range(NQ):
            f = (b * NQ + q) % INPUT_DIM
            enc_h[:, b, q] = 0.5 * ENC_LAMBDA * (alpha[f] * x[:, f] + beta[f])
    th_h = 0.5 * theta  # (30,)

    ca, sa = np.cos(enc_h[:, 0, :]), np.sin(enc_h[:, 0, :])
    c2, s2 = np.cos(th_h[:NQ]), np.sin(th_h[:NQ])
    v0 = c2 * ca - 1j * s2 * sa
    v1 = c2 * sa - 1j * s2 * ca

    # level-4 product state (qubits 0..4, 32 amps) computed host-side; the
    # device build then only expands qubits 5..9. Layout: 32 real cols then
    # 32 imag cols (matches the device state layout), DMA'd straight into
    # the build buffer. Amp index: qubit 0 = MSB of the 5-bit index.
    lvl = np.ones((B, 1), np.complex128)
    for q in range(5):
        vq = np.stack([v0[:, q], v1[:, q]], axis=1)  # (B, 2)
        lvl = (lvl[:, :, None] * vq[:, None, :]).reshape(B, -1)
    cols = [lvl[:, j].real for j in range(32)] + [lvl[:, j].imag for j in range(32)]
    for q in range(5, NQ):
        cols += [v0[:, q].real, v1[:, q].real, v0[:, q].imag, v1[:, q].imag]
    for b in (1, 2):  # ENC: (c, s, -s) per (layer, qubit) -> cols 84..143
        for q in range(NQ):
            c_, s_ = np.cos(enc_h[:, b, q]), np.sin(enc_h[:, b, q])
            cols += [c_, s_, -s_]
    for b in (1, 2):  # RX: (tan, -tan) of half angle -> cols 144..183... base 144
        for q in range(NQ):
            t_ = np.tan(th_h[NQ * b + q])
            cols += [np.full(B, t_), np.full(B, -t_)]
    params = np.stack(cols, axis=1).astype(np.float32)  # (B, 184)

    i = np.arange(NSTATE)
    W0 = np.zeros(NSTATE, np.float64)
    hw = np.asarray(head_w, np.float64)
    for q in range(NQ):
        W0 += hw[0, q] * (1.0 - 2.0 * ((i >> (NQ - 1 - q)) & 1))
    bits = (i[:, None] >> (NQ - 1 - np.arange(NQ))[None, :]) & 1
    nb = bits.copy()
    for c in range(NQ - 1):
        nb[:, c + 1] ^= nb[:, c]
    Pperm = (nb * (1 << (NQ - 1 - np.arange(NQ)))[None, :]).sum(1)
    scale = float(np.clip(np.asarray(logit_scale, np.float64),
                          LOGIT_SCALE_MIN, LOGIT_SCALE_MAX))
    W = (scale * (W0[Pperm] + float(np.asarray(head_b).ravel()[0])))
    # RX gates run as tan-shears: each drops a cos(half) factor, uniform
    # across samples (theta shared). probs scale by (prod cos)^2 -> fold here.
    W = W * float(np.prod(np.cos(th_h[NQ:3 * NQ])) ** 2)
    return params, W.astype(np.float32)


# ------------------------------------------------------------- device build
class _Sched:
    """Static engine round-robins, tunable."""

    def __init__(self, nc):
        self.nc = nc
        self.act, self.dve, self.pool = nc.scalar, nc.vector, nc.gpsimd
        self._ts = 0
        self._stt = 0
        self._cp = 0
        self._small = 0

    # out = in0 * scalar  (scalar: AP (128,1) or float)
    def ts(self, out, in0, scalar):
        self._emit_ts(self.act, out, in0, scalar)

    def _emit_ts(self, eng, out, in0, scalar):
        if eng is self.act:
            eng.mul(out, in0, scalar)
        else:
            eng.tensor_scalar_mul(out, in0, scalar)

    # out = (in0 * scalar) op1 in1 — DVE only (Pool HW rejects TensorScalarPtr)
    def stt(self, out, in0, scalar, in1, sub):
        from concourse import mybir
        self.dve.scalar_tensor_tensor(
            out, in0, scalar, in1, mybir.AluOpType.mult,
            mybir.AluOpType.subtract if sub else mybir.AluOpType.add)

    def cp(self, out, in0):
        pick = [self.act, self.pool][self._cp % 2] if OPTS["pool_copies"] else self.act
        self._cp += 1
        if pick is self.act:
            pick.copy(out, in0)
        else:
            pick.tensor_copy(out=out, in_=in0)

    def tt_add(self, out, in0, in1):  # Pool elementwise add (TT ucode)
        self.pool.tensor_add(out=out, in0=in0, in1=in1)

    # small build TS: ACT only (keeps build fan-in low)
    def small_ts(self, out, in0, scalar):
        self._emit_ts(self.act, out, in0, scalar)


def _emit_tile_gen(nc, tc, pool, t, params_d, wv, out_d):
    from concourse import mybir
    f32 = mybir.dt.float32
    Alu = mybir.AluOpType
    sch = _Sched(nc)

    par = pool.tile([P, PCOLS], f32, name=f"par{t}")
    nc.sync.dma_start(out=par[:, :], in_=params_d[t])
    A = pool.tile([P, 2 * NSTATE], f32, name=f"A{t}")
    B = pool.tile([P, 2 * NSTATE], f32, name=f"B{t}")
    scr = pool.tile([P, 2 * NSTATE], f32, name=f"scr{t}")

    def col(j):
        return par[:, j:j + 1]

    # ---- layer-1 product state: level-4 (32 amps) DMA'd from the host
    # params block, then 5 expansions (q=5..9), ping-pong B -> A -> ... -> A.
    cur, nxt = B, A
    nc.sync.dma_start(out=cur[:, 0:32], in_=params_d[t][:, 0:32])
    nc.sync.dma_start(out=cur[:, NSTATE:NSTATE + 32], in_=params_d[t][:, 32:64])

    def vcol(q, j):
        return col(64 + 4 * (q - 5) + j)

    for q in range(5, NQ):
        n = 1 << q
        ar, ai = cur[:, 0:n], cur[:, NSTATE:NSTATE + n]
        for b in (0, 1):
            vr, vi = vcol(q, b), vcol(q, 2 + b)
            out_r = nxt[:, b:2 * n:2]
            out_i = nxt[:, NSTATE + b:NSTATE + 2 * n:2]
            sr = scr[:, 512 * b:512 * b + n]
            si = scr[:, NSTATE + 512 * b:NSTATE + 512 * b + n]
            sch.small_ts(sr, ai, vi)
            sch.stt(out_r, ar, vr, sr, sub=True)       # ar*vr - ai*vi
            sch.small_ts(si, ai, vr)
            sch.stt(out_i, ar, vi, si, sub=False)      # ar*vi + ai*vr
        cur, nxt = nxt, cur
        yield
    S, T = cur, nxt  # state / scratch (5 expansions: ends in A... tracked)

    # HW APs allow at most 3 free dims. The plane dim (stride 1024) folds with
    # the hi dim (count 2^q, stride 2^(10-q)) into one "ph" dim of 2^(q+1).
    def view3(buf, q):
        return buf[:, :].rearrange("p (ph b lo) -> p ph b lo",
                                   ph=1 << (q + 1), b=2, lo=1 << (NQ - 1 - q))

    def plane_view3(buf, plane, q):  # one plane, [hi, b, lo]
        pl = buf[:, NSTATE * plane:NSTATE * (plane + 1)]
        return pl.rearrange("p (hi b lo) -> p hi b lo",
                            hi=1 << q, b=2, lo=1 << (NQ - 1 - q))

    enc_n = [0]

    def rot_ry(q, ccol, scol, nscol):
        # ENC: T[bit0] = -s*s1 ; T[bit1] = +s*s0 (ACT), then combine.
        # Default: in-place S = S*c + T as one 2048-elem DVE STT. Every
        # OPTS['enc_divert']-th gate instead materializes w = c*S on ACT and
        # adds on Pool (2 TT-1024), relieving the DVE bottleneck.
        Sv, Tv = view3(S, q), view3(T, q)
        sch.ts(Tv[:, :, 0, :], Sv[:, :, 1, :], nscol)
        sch.ts(Tv[:, :, 1, :], Sv[:, :, 0, :], scol)
        enc_n[0] += 1
        k = OPTS.get("enc_divert", 0)
        if k and enc_n[0] % k == 0:
            sch.ts(scr[:, :], S[:, :], ccol)              # ACT: w = c*S
            sch.tt_add(S[:, 0:NSTATE], scr[:, 0:NSTATE], T[:, 0:NSTATE])
            sch.tt_add(S[:, NSTATE:], scr[:, NSTATE:], T[:, NSTATE:])
        else:
            sch.stt(S[:, :], S[:, :], ccol, T[:, :], sub=False)

    def rot_rx_shear(q, tcol, ntcol):
        # theta-RX tan-shear: T_r = t*swap(S_i) + S_r ; T_i = -t*swap(S_r)+S_i
        # cos factor folded into W host-side. Buffer swap. STT allows only 2
        # free dims -> split by (plane, bit): 4 DVE STTs of 512.
        nonlocal S, T
        Tv = [plane_view3(T, p_, q) for p_ in (0, 1)]
        Sv = [plane_view3(S, p_, q) for p_ in (0, 1)]
        for b in (0, 1):
            sch.stt(Tv[0][:, :, b, :], Sv[1][:, :, 1 - b, :], tcol,
                    Sv[0][:, :, b, :], sub=False)
            sch.stt(Tv[1][:, :, b, :], Sv[0][:, :, 1 - b, :], ntcol,
                    Sv[1][:, :, b, :], sub=False)
        S, T = T, S

    def rot_rx_divert(q, tcol, ntcol):
        # same shear but u = t*swap(S) on ACT, then in-place Pool TT adds.
        sch.ts(plane_view3(T, 0, q), plane_view3(S, 1, q)[:, :, ::-1, :], tcol)
        sch.ts(plane_view3(T, 1, q), plane_view3(S, 0, q)[:, :, ::-1, :], ntcol)
        sch.tt_add(S[:, 0:NSTATE], S[:, 0:NSTATE], T[:, 0:NSTATE])
        sch.tt_add(S[:, NSTATE:], S[:, NSTATE:], T[:, NSTATE:])

    def cnot_pair(c):
        # CNOT(c,c+1) then CNOT(c+1,c+2): out[B0,B1,B2]=in[B0,B1^B0,B2^B1].
        # 4 strided copies of 512, then swap buffer roles.
        nonlocal S, T
        def v(buf):
            return buf[:, :].rearrange(
                "p (ph bc b1 b2 lo) -> p ph bc b1 b2 lo",
                ph=1 << (c + 1), bc=2, b1=2, b2=2, lo=1 << (NQ - 3 - c))
        Sv, Tv = v(S), v(T)
        for B0 in (0, 1):
            for B1 in (0, 1):
                outv = Tv[:, :, B0, B1, :, :]
                inv = Sv[:, :, B0, B1 ^ B0, :, :]
                if B1:
                    inv = Sv[:, :, B0, B1 ^ B0, ::-1, :]
                sch.cp(outv, inv)
        S, T = T, S

    def cnot(c):
        nonlocal S, T
        def v6(buf):
            return buf[:, :].rearrange(
                "p (ph bc bt lo) -> p ph bc bt lo",
                ph=1 << (c + 1), bc=2, bt=2, lo=1 << (NQ - 2 - c))
        Sv, Tv = v6(S), v6(T)
        sch.cp(Tv[:, :, 0, :, :], Sv[:, :, 0, :, :])          # identity half
        sch.cp(Tv[:, :, 1, :, :], Sv[:, :, 1, ::-1, :])       # flipped half
        S, T = T, S

    def cnot_triple(c):
        # CNOT(c,c+1);CNOT(c+1,c+2);CNOT(c+2,c+3):
        # out[B0,B1,B2,B3] = in[B0, B1^B0, B2^B1, B3^B2]. 8 copies of 256.
        nonlocal S, T
        def v(buf):
            return buf[:, :].rearrange(
                "p (ph bc b1 b2 b3 lo) -> p ph bc b1 b2 b3 lo",
                ph=1 << (c + 1), bc=2, b1=2, b2=2, b3=2, lo=1 << (NQ - 4 - c))
        Sv, Tv = v(S), v(T)
        for B0 in (0, 1):
            for B1 in (0, 1):
                for B2 in (0, 1):
                    outv = Tv[:, :, B0, B1, B2, :, :]
                    if B2:
                        inv = Sv[:, :, B0, B1 ^ B0, B2 ^ B1, ::-1, :]
                    else:
                        inv = Sv[:, :, B0, B1 ^ B0, B2 ^ B1, :, :]
                    sch.cp(outv, inv)
        S, T = T, S

    def chain():
        if OPTS["triples"]:
            for c in (0, 3, 6):
                cnot_triple(c)
                yield
        else:
            for c in (0, 2, 4, 6):
                cnot_pair(c)
                yield
            cnot(8)
            yield

    def enc_col(lay, q):
        return 84 + 30 * lay + 3 * q

    def rx_col(lay, q):
        return 144 + 20 * lay + 2 * q

    rx_divert = 0
    yield from chain()
    for lay in (0, 1):
        for q in range(NQ):
            j = enc_col(lay, q)
            rot_ry(q, col(j), col(j + 1), col(j + 2))
            yield
        for q in range(NQ):
            j = rx_col(lay, q)
            if rx_divert % 5 < OPTS["divert"]:         # some to ACT+Pool
                rot_rx_divert(q, col(j), col(j + 1))
            else:
                rot_rx_shear(q, col(j), col(j + 1))
            rx_divert += 1
            yield
        if lay == 0:
            yield from chain()

    # ---- probs + folded-head weighted reduce
    res = pool.tile([P, 1], f32, name=f"res{t}")
    Pr, Pi = T[:, 0:NSTATE], T[:, NSTATE:]
    nc.scalar.square(Pr, S[:, 0:NSTATE])
    nc.scalar.square(Pi, S[:, NSTATE:])
    nc.vector.tensor_add(out=Pr, in0=Pr, in1=Pi)
    # (P*1.0) elementwise-mult W, accum_out = sum -> raw logit. (TTR compiles
    # but faults at runtime in this environment; STT+accum_out is equivalent.)
    nc.vector.scalar_tensor_tensor(
        Pi, Pr, 1.0, wv[:, :], Alu.mult, Alu.mult, accum_out=res[:, :])
    nc.sync.dma_start(out=out_d[t], in_=res[:, :])


def build_module():
    from concourse import bacc, mybir, tile
    f32 = mybir.dt.float32
    nc = bacc.Bacc(None, target_bir_lowering=False)
    params_d = nc.dram_tensor("params", [TILES, P, PCOLS], f32,
                              kind="ExternalInput")
    wvec_d = nc.dram_tensor("wvec", [P, NSTATE], f32, kind="ExternalInput")
    out_d = nc.dram_tensor("out", [TILES, P, 1], f32, kind="ExternalOutput")
    with tile.TileContext(nc) as tc:
        with tc.tile_pool(name="main", bufs=1) as pool:
            wv = pool.tile([P, NSTATE], f32, name="wv")
            nc.sync.dma_start(out=wv[:, :], in_=wvec_d[:, :])
            # interleave the two tiles' emission per phase so the
            # scheduler overlaps tile0's DVE combines with tile1's ACT stage
            gens = [_emit_tile_gen(nc, tc, pool, t, params_d, wv, out_d)
                    for t in range(TILES)]
            live = list(gens)
            while live:
                nxt_live = []
                for g in live:
                    try:
                        next(g)
                        nxt_live.append(g)
                    except StopIteration:
                        pass
                live = nxt_live
    nc.compile()  # Bacc pipeline: splits >1-wait instrs into event semaphores
    return nc


# ------------------------------------------------------------------ runner
def _get_module():
    if "nc" not in _CACHE:
        _CACHE["nc"] = build_module()
    return _CACHE["nc"]


def make_in_maps(params, W):
    """params (2048,120), W (1024,) -> list of 8 per-core input dicts."""
    pt = np.ascontiguousarray(params.reshape(N_CORES, TILES, P, PCOLS))
    wb = np.ascontiguousarray(np.broadcast_to(W, (P, NSTATE)))
    return [{"params": pt[k], "wvec": wb} for k in range(N_CORES)]


def kernel(x, theta, enc_alpha_raw, enc_beta_raw, head_w, head_b, logit_scale):
    params, W = host_precompute(x, theta, enc_alpha_raw, enc_beta_raw,
                                head_w, head_b, logit_scale)
    nc = _get_module()
    from concourse.bass_utils import run_bass_kernel_spmd
    res = run_bass_kernel_spmd(nc, make_in_maps(params, W),
                               core_ids=list(range(N_CORES)))
    out = np.concatenate([r["out"].reshape(TILES * P, 1) for r in res.results])
    return np.clip(out, -LOGIT_CLAMP, LOGIT_CLAMP).astype(np.float32)

